# revision 1
# baseline (speedup 1.0000x reference)
"""Trainium2 Bass kernel for nn_Attention_org_single_85074712199391.

Channel-attention module. Reference math (per batch b, head h):
    Qc = emb1[b].reshape(N, 4, dq)[:, h]          # [N, 128]
    Kc = emb_all[b].reshape(N, 4, dk)[:, h]       # [N, 240]
    Q = Qc @ Wq[h].T ; K = Kc @ Wk.T ; V = Kc @ Wv.T
    scores = Q.T @ K / sqrt(KV)                   # [128, 240]
    probs = softmax(instnorm(scores), axis=-1)
    context = probs @ V.T                         # [128, N]
    O1 = permute/concat(context) @ Wo.T           # [N, 512]

Algebraic rewrite used here (exact):
    S_h      = Qc.T @ Kc                          # big contraction over N
    scores_h = (Wq[h]/sqrt(KV)) @ S_h @ Wk.T
    probs_h  = softmax over dk of rstd*scores_h   # mean cancels in softmax
    P2_h     = probs_h @ Wv                       # [128, 240]
    ctx_h    = P2_h @ Kc.T                        # [128, N]
    O1       = sum_h ctx_h.T @ Wo[:, h::4].T     # accumulate over heads

Phases per core (core b owns batch b; weights replicated; no collectives):
    A: stream e1/ea as bf16 (DMA-cast loads); accumulate S_h in PSUM and
       PE-transpose ea into a resident bf16 eaT for phase C.
    B: tiny fp32 scores path, instance-norm stats via ones-matmul
       (partition reduce + broadcast in one op), batched softmax across
       heads, P2.T in bf16.
    C: context matmuls off eaT, then the output projection accumulating
       over heads with full-128 contractions; store fp32.
"""

import sys

import numpy as np

try:
    import concourse.bass as bass
except ImportError:  # harness environments without the repo on sys.path
    sys.path.insert(0, "/opt/trn_rl_repo")
    import concourse.bass as bass

import concourse.bacc as bacc

import ml_dtypes
import concourse.mybir as mybir
import concourse.tile as tile
from concourse.bass_utils import run_bass_kernel_spmd

F32 = mybir.dt.float32
BF16 = mybir.dt.bfloat16
AF = mybir.ActivationFunctionType
ALU = mybir.AluOpType

B, N, C, KV, H = 8, 4096, 512, 960, 4
DQ, DK = C // 4, KV // 4          # 128, 240
PT = 128                          # partition tile
NT = N // PT                      # 32 row tiles
NCH = N // 512                    # 8 column chunks for phase C
DCH = 8                           # KV split into 8 chunks of 120 partitions
CHW = KV // DCH                   # 120
KCH = 2                           # dk split for 240-deep contractions
KHW = DK // KCH                   # 120
EPS = 1e-5
NORM_CNT = float(DQ * DK)         # instance-norm element count

import os as _os
PA_BUFS = int(_os.environ.get("PA_BUFS", "4"))
PST_BUFS = int(_os.environ.get("PST_BUFS", "3"))
PSB_BUFS = int(_os.environ.get("PSB_BUFS", "4"))
CX_BUFS = int(_os.environ.get("CX_BUFS", "3"))
O_BUFS = int(_os.environ.get("O_BUFS", "3"))
EVAC_SPLIT = _os.environ.get("EVAC_SPLIT", "0") == "1"


def build_nc(ablate=frozenset(), reps=1):
    nc = bacc.Bacc("TRN2", target_bir_lowering=False, debug=False)

    e1 = nc.dram_tensor("e1", [N, C], F32, kind="ExternalInput").ap()
    ea = nc.dram_tensor("ea", [N, KV], F32, kind="ExternalInput").ap()
    wqt = nc.dram_tensor("wqt", [DQ, H, DQ], F32, kind="ExternalInput").ap()
    wkt = nc.dram_tensor("wkt", [DK, DK], F32, kind="ExternalInput").ap()
    wvb = nc.dram_tensor("wvb", [DK, DK], BF16, kind="ExternalInput").ap()
    wotb = nc.dram_tensor("wotb", [DQ, H, C], BF16, kind="ExternalInput").ap()
    idf = nc.dram_tensor("idf", [PT, PT], F32, kind="ExternalInput").ap()
    idb = nc.dram_tensor("idb", [PT, PT], BF16, kind="ExternalInput").ap()
    o1 = nc.dram_tensor("o1", [N, C], F32, kind="ExternalOutput").ap()

    with tile.TileContext(nc) as tc:
        for _ in range(reps):
            build_body(tc, e1, ea, wqt, wkt, wvb, wotb, idf, idb, o1, ablate)
    # Legalize: walrus encodes at most one sync-wait command per instruction;
    # bacc's compile() splits excess waits into event-semaphore instructions.
    nc.compile()
    return nc


def build_body(tc, e1, ea, wqt, wkt, wvb, wotb, idf, idb, o1, ablate=frozenset()):
    nc = tc.nc
    from contextlib import ExitStack, nullcontext

    with ExitStack() as stk:
        pW = stk.enter_context(tc.tile_pool(name="persist", bufs=1))

        # --- persistent weights / constants -----------------------------------
        wqt_sb = pW.tile([DQ, H, DQ], F32, tag="wqt_sb")
        nc.sync.dma_start(wqt_sb[:], wqt[:])
        wkt_sb = pW.tile([KHW, KCH, DK], F32, tag="wkt_sb")
        wvb_sb = pW.tile([KHW, KCH, DK], BF16, tag="wvb_sb")
        for j in range(KCH):
            nc.sync.dma_start(wkt_sb[:, j, :], wkt[j * KHW:(j + 1) * KHW, :])
            nc.sync.dma_start(wvb_sb[:, j, :], wvb[j * KHW:(j + 1) * KHW, :])
        wotb_sb = pW.tile([DQ, H, C], BF16, tag="wotb_sb")
        nc.sync.dma_start(wotb_sb[:], wotb[:])
        idf_sb = pW.tile([PT, PT], F32, tag="idf_sb")
        nc.sync.dma_start(idf_sb[:], idf[:])
        idb_sb = pW.tile([PT, PT], BF16, tag="idb_sb")
        nc.sync.dma_start(idb_sb[:], idb[:])
        ones_sb = pW.tile([PT, PT], F32, tag="ones_sb")
        nc.vector.memset(ones_sb[:], 1.0)
        eps_sb = pW.tile([PT, 1], F32, tag="eps_sb")
        nc.vector.memset(eps_sb[:], EPS)

        # --- persistent activations -------------------------------------------
        eaT_sb = pW.tile([CHW, DCH, N], BF16, tag="eaT_sb")    # ea transposed
        s_sb = pW.tile([DQ, H, DK], F32, tag="s_sb")           # S_h
        sc_all = pW.tile([DQ, H, DK], F32, tag="sc_all")       # scores
        e_all = pW.tile([DQ, H, DK], BF16, tag="e_all")        # exp()
        stats = pW.tile([DQ, H, 2], F32, tag="stats")          # row sums, sq
        p2t_sb = pW.tile([KHW, H, 2, DQ], BF16, tag="p2t_sb")  # P2.T chunks

        def small(name):
            return pW.tile([DQ, H], F32, tag=name, name=name)

        mu_all = small("mu_all")
        m2_all = small("m2_all")
        mu2_all = small("mu2_all")
        var_all = small("var_all")
        sd_all = small("sd_all")
        rstd_all = small("rstd_all")
        den_all = small("den_all")
        rec_all = small("rec_all")

        pA = stk.enter_context(tc.tile_pool(name="pA", bufs=PA_BUFS))
        pBs = stk.enter_context(tc.tile_pool(name="pBs", bufs=2))
        pC = stk.enter_context(tc.tile_pool(name="pC", bufs=4))

        # --- phase A: bf16 DMA-cast loads; S accumulation; eaT transposes -----
        with tc.tile_pool(name="psS", bufs=1, space="PSUM") as psS, \
             (tc.tile_pool(name="psT", bufs=PST_BUFS, space="PSUM")
              if "tpose" not in ablate else nullcontext()) as psT:
            s_ps = [psS.tile([DQ, DK], F32, tag=f"s{h}", name=f"s_ps{h}")
                    for h in range(H)]
            for ii in range(NT // 4):
                # two 128-row tiles per DMA: row a*128+p of the pair lands on
                # partition p, free-slot a (fewer, larger DMA transfers)
                prows = slice(ii * 4 * PT, (ii + 1) * 4 * PT)
                e1b = pA.tile([PT, 4, C], BF16, tag="e1b")
                nc.gpsimd.dma_start(
                    e1b[:], e1[prows, :].rearrange("(a p) k -> p a k", p=PT))
                eab = pA.tile([PT, 4, KV], BF16, tag="eab")
                nc.gpsimd.dma_start(
                    eab[:], ea[prows, :].rearrange("(a p) k -> p a k", p=PT))
                for a in range(4):
                    i = 4 * ii + a
                    rows = slice(i * PT, (i + 1) * PT)
                    if "sA" not in ablate:
                        for h in range(H):
                            nc.tensor.matmul(
                                s_ps[h][:],
                                e1b[:, a, h * DQ:(h + 1) * DQ],
                                eab[:, a, h * DK:(h + 1) * DK],
                                start=(i == 0),
                                stop=(i == NT - 1),
                            )
                    if "tpose" not in ablate:
                        for j in range(DCH):
                            tp = psT.tile([CHW, PT], BF16, tag="tp", name="tp")
                            nc.tensor.transpose(
                                tp[:], eab[:, a, j * CHW:(j + 1) * CHW],
                                idb_sb[:])
                            nc.vector.tensor_copy(eaT_sb[:, j, rows], tp[:])
            if "sA" not in ablate:
                for h in range(H):
                    nc.scalar.copy(s_sb[:, h, :], s_ps[h][:])

        # --- phase B -----------------------------------------------------------
        with (tc.tile_pool(name="psB", bufs=PSB_BUFS, space="PSUM")
              if "phaseB" not in ablate else nullcontext()) as psB:
            for h in range(H if "phaseB" not in ablate else 0):
                # U.T = (S.T-chunks) @ (Wq_h.T/sqrt(KV))  [240k, 128e], fp32
                # (computed directly in transposed form: S as the stationary
                # operand; avoids a PE transpose round-trip through PSUM)
                ut_sb = pBs.tile([KHW, KCH, DQ], F32, tag="ut_sb", name="ut_sb")
                for j in range(KCH):
                    ut_ps = psB.tile([KHW, DQ], F32, tag="psb", name="ut_ps")
                    nc.tensor.matmul(ut_ps[:],
                                     s_sb[:, h, j * KHW:(j + 1) * KHW],
                                     wqt_sb[:, h, :], start=True, stop=True)
                    nc.vector.tensor_copy(ut_sb[:, j, :], ut_ps[:])
                # scores = U @ Wk.T  [128e, 240ek], fp32
                sc_ps = psB.tile([DQ, DK], F32, tag="psb", name="sc_ps")
                for j in range(KCH):
                    nc.tensor.matmul(sc_ps[:], ut_sb[:, j, :], wkt_sb[:, j, :],
                                     start=(j == 0), stop=(j == KCH - 1))
                # evacuate + per-row sums of x and x^2 for instance-norm
                nc.scalar.activation(sc_all[:, h, :], sc_ps[:], AF.Copy,
                                     accum_out=stats[:, h, 0:1])
                junk = pBs.tile([DQ, DK], F32, tag="junk", name="junk")
                nc.scalar.activation(junk[:], sc_ps[:], AF.Square,
                                     accum_out=stats[:, h, 1:2])

            if "phaseB" not in ablate:
                # cross-partition reduce of stats; every partition gets totals
                tot_ps = psB.tile([DQ, H, 2], F32, tag="psb", name="tot_ps")
                nc.tensor.matmul(tot_ps[:], ones_sb[:], stats[:],
                                 start=True, stop=True)
                nc.scalar.mul(mu_all[:], tot_ps[:, :, 0:1], 1.0 / NORM_CNT)
                nc.scalar.mul(m2_all[:], tot_ps[:, :, 1:2], 1.0 / NORM_CNT)
                nc.scalar.square(mu2_all[:], mu_all[:])
                nc.vector.tensor_sub(var_all[:], m2_all[:], mu2_all[:])
                nc.scalar.activation(sd_all[:], var_all[:], AF.Sqrt,
                                     bias=eps_sb[:, 0:1])
                nc.vector.reciprocal(rstd_all[:], sd_all[:])
                # softmax over ek of rstd*scores: the mean shift cancels in
                # softmax, and no max-shift is needed -- scores are z-scored
                # by rstd so |exponent| stays ~<=8, far from fp32 overflow.
                # The 1/denominator is applied later as the ctx-evac scale.
                for h in range(H):
                    nc.scalar.activation(e_all[:, h, :], sc_all[:, h, :],
                                         AF.Exp, scale=rstd_all[:, h:h + 1],
                                         accum_out=den_all[:, h:h + 1])
                nc.vector.reciprocal(rec_all[:], den_all[:])
                # P2.T = (exp @ Wv).T in chunks: [120d, 128c]  (unnormalized)
                for h in range(H):
                    pt_sb = pBs.tile([KHW, KCH, DQ], BF16, tag="pt_sb",
                                     name="pt_sb")
                    for j in range(KCH):
                        pt_ps = psB.tile([KHW, DQ], BF16, tag="psb",
                                         name="pt_ps")
                        nc.tensor.transpose(
                            pt_ps[:], e_all[:, h, j * KHW:(j + 1) * KHW],
                            idb_sb[:])
                        nc.vector.tensor_copy(pt_sb[:, j, :], pt_ps[:])
                    for jd in range(2):
                        p2t_ps = psB.tile([CHW, DQ], F32, tag="psb",
                                          name="p2t_ps")
                        for jk in range(KCH):
                            nc.tensor.matmul(
                                p2t_ps[:],
                                wvb_sb[:, jk, jd * CHW:(jd + 1) * CHW],
                                pt_sb[:, jk, :],
                                start=(jk == 0), stop=(jk == KCH - 1))
                        nc.scalar.copy(p2t_sb[:, h, jd, :], p2t_ps[:])

        # --- phase C: ctx_h = P2_h @ Kc_h.T, then O1 = sum_h ctx_h.T @ WoT_h --
        # Software-pipelined: the context matmuls for chunk n+1 are emitted
        # before chunk n's output projection, so the PE never stalls on the
        # scalar-engine context evacuations.
        with (tc.tile_pool(name="psC", bufs=CX_BUFS, space="PSUM")
              if "phaseC" not in ablate else nullcontext()) as psC:

            def emit_ctx(nch):
                ncols = slice(nch * 512, (nch + 1) * 512)
                ctx = pC.tile([DQ, H, 512], BF16, tag="ctx", name="ctx", bufs=3)
                for h in range(H):
                    cx_ps = psC.tile([DQ, 512], F32, tag="cx", name="cx_ps", bufs=CX_BUFS)
                    for jd in range(2):
                        nc.tensor.matmul(cx_ps[:], p2t_sb[:, h, jd, :],
                                         eaT_sb[:, 2 * h + jd, ncols],
                                         start=(jd == 0), stop=(jd == 1))
                    # normalize: rows of ctx are q-channels, so the softmax
                    # denominator applies as a per-partition scale here.
                    # On DVE so it runs parallel to ACT's output evacuations.
                    nc.vector.tensor_scalar_mul(ctx[:, h, :], cx_ps[:],
                                                rec_all[:, h:h + 1])
                return ctx

            def emit_oproj(nch, ctx):
                for t in range(4):
                    i = nch * 4 + t
                    rows = slice(i * PT, (i + 1) * PT)
                    o_ps = psC.tile([PT, C], F32, tag="o", name="o_ps", bufs=O_BUFS)
                    for h in range(H):
                        nc.tensor.matmul(o_ps[:],
                                         ctx[:, h, t * PT:(t + 1) * PT],
                                         wotb_sb[:, h, :],
                                         start=(h == 0), stop=(h == H - 1))
                    o_sb = pC.tile([PT, C], F32, tag="o_sb", name="o_sb")
                    nc.scalar.copy(o_sb[:], o_ps[:])
                    nc.sync.dma_start(o1[rows, :], o_sb[:])

            if "phaseC" not in ablate:
                prev = emit_ctx(0)
                for nch in range(1, NCH):
                    cur = emit_ctx(nch)
                    emit_oproj(nch - 1, prev)
                    prev = cur
                emit_oproj(NCH - 1, prev)


_NC_CACHE = None


def get_nc():
    global _NC_CACHE
    if _NC_CACHE is None:
        _NC_CACHE = build_nc()
    return _NC_CACHE


def make_in_maps(emb1, emb_all, Wq, Wk, Wv, Wo):
    emb1 = np.ascontiguousarray(np.asarray(emb1, dtype=np.float32))
    emb_all = np.ascontiguousarray(np.asarray(emb_all, dtype=np.float32))
    Wq = np.asarray(Wq, dtype=np.float32)
    Wk = np.asarray(Wk, dtype=np.float32)
    Wv = np.asarray(Wv, dtype=np.float32)
    Wo = np.asarray(Wo, dtype=np.float32)

    scale = 1.0 / np.sqrt(np.float32(KV))
    wqt_np = np.ascontiguousarray(np.transpose(Wq, (2, 0, 1)) * scale)  # [c,h,e]
    wkt_np = np.ascontiguousarray(Wk.T)                                 # [k,ek]
    wvb_np = np.ascontiguousarray(Wv).astype(ml_dtypes.bfloat16)        # [k,d]
    wotb_np = np.ascontiguousarray(
        Wo.reshape(C, DQ, H).transpose(1, 2, 0)).astype(ml_dtypes.bfloat16)
    idf_np = np.eye(PT, dtype=np.float32)
    idb_np = np.eye(PT, dtype=ml_dtypes.bfloat16)

    shared = {"wqt": wqt_np, "wkt": wkt_np, "wvb": wvb_np, "wotb": wotb_np,
              "idf": idf_np, "idb": idb_np}
    return [
        {"e1": emb1[b], "ea": emb_all[b], **shared}
        for b in range(B)
    ]


def run(inputs, trace=False, **spmd_kwargs):
    nc = get_nc()
    in_maps = make_in_maps(**inputs)
    res = run_bass_kernel_spmd(nc, in_maps, list(range(B)), trace=trace,
                               **spmd_kwargs)
    out = np.stack([np.asarray(res.results[b]["o1"]) for b in range(B)], axis=0)
    return out.astype(np.float32, copy=False), res


def kernel(**inputs) -> np.ndarray:
    out, _ = run(inputs, trace=False)
    return out



# revision 14
# speedup vs baseline: 1.3512x; 1.3512x over previous
"""Trainium2 Bass kernel for nn_Attention_org_single_85074712199391.

Channel-attention module. Reference math (per batch b, head h):
    Qc = emb1[b].reshape(N, 4, dq)[:, h]          # [N, 128]
    Kc = emb_all[b].reshape(N, 4, dk)[:, h]       # [N, 240]
    Q = Qc @ Wq[h].T ; K = Kc @ Wk.T ; V = Kc @ Wv.T
    scores = Q.T @ K / sqrt(KV)                   # [128, 240]
    probs = softmax(instnorm(scores), axis=-1)
    context = probs @ V.T                         # [128, N]
    O1 = permute/concat(context) @ Wo.T           # [N, 512]

Algebraic rewrite used here (exact):
    S_h      = Qc.T @ Kc                          # big contraction over N
    scores_h = (Wq[h]/sqrt(KV)) @ S_h @ Wk.T
    probs_h  = softmax over dk of rstd*scores_h   # mean cancels in softmax
    P2_h     = probs_h @ Wv                       # [128, 240]
    ctx_h    = P2_h @ Kc.T                        # [128, N]
    O1       = sum_h ctx_h.T @ Wo[:, h::4].T      # accumulate over heads

Sharding: core b owns batch b; weights replicated; no collectives.

Pipelined-reps structure: ALL tile pools are created once at TileContext
scope and shared by every rep body.  Per-rep tiles are requested with a
fixed tag, so the pool's buf rotation gives automatic ping/pong: tensors
that are still being read at the end of rep i (eaT, p2t, rec) use bufs=2,
which makes rep i+1's writes land in the other buffer and lets rep i+1's
load phase stream underneath rep i's compute tail.  Weights are loaded
once, outside the rep loop.

Engine budget per rep (steady state):
    DMA : e1+ea fp32 reads (23.7 MB) + o1 writes (8 MB)  -- the roofline
    PE  : S matmuls, ea transposes, tiny phase B, ctx + output projection
    DVE : half the eaT evacuations, ctx evacuations, phase-B smalls,
          bit-trick rsqrt (keeps ACT pinned to the exp_and_others table)
    ACT : other half of eaT evacuations, o evacuations, stats + exp
    POOL: SWDGE descriptor generation for the cast loads (f32->bf16)
"""

import sys

import numpy as np

try:
    import concourse.bass as bass
except ImportError:  # harness environments without the repo on sys.path
    sys.path.insert(0, "/opt/trn_rl_repo")
    import concourse.bass as bass

import concourse.bacc as bacc

import ml_dtypes
import concourse.mybir as mybir
import concourse.tile as tile
from concourse.bass_utils import run_bass_kernel_spmd

F32 = mybir.dt.float32
BF16 = mybir.dt.bfloat16
I32 = mybir.dt.int32
AF = mybir.ActivationFunctionType
ALU = mybir.AluOpType

B, N, C, KV, H = 8, 4096, 512, 960, 4
DQ, DK = C // 4, KV // 4          # 128, 240
PT = 128                          # partition tile
NT = N // PT                      # 32 row tiles
GRP = 4                           # row tiles per DMA group
NG = NT // GRP                    # 8 groups
DCH = 8                           # KV split into 8 chunks of 120 partitions
CHW = KV // DCH                   # 120
KCH = 2                           # dk split for 240-deep contractions
KHW = DK // KCH                   # 120
NCH = N // 512                    # 8 column chunks for phase C
EPS = 1e-5
NORM_CNT = float(DQ * DK)         # instance-norm element count
RSQRT_MAGIC = 0x5F3759DF


def build_nc(reps=1):
    from contextlib import ExitStack

    nc = bacc.Bacc("TRN2", target_bir_lowering=False, debug=False)

    e1 = nc.dram_tensor("e1", [N, C], F32, kind="ExternalInput").ap()
    ea = nc.dram_tensor("ea", [N, KV], F32, kind="ExternalInput").ap()
    wqt = nc.dram_tensor("wqt", [DQ, H, DQ], F32, kind="ExternalInput").ap()
    wkt = nc.dram_tensor("wkt", [DK, DK], F32, kind="ExternalInput").ap()
    wvb = nc.dram_tensor("wvb", [DK, DK], BF16, kind="ExternalInput").ap()
    wotb = nc.dram_tensor("wotb", [DQ, H, C], BF16, kind="ExternalInput").ap()
    idb = nc.dram_tensor("idb", [PT, PT], BF16, kind="ExternalInput").ap()
    o1 = nc.dram_tensor("o1", [N, C], F32, kind="ExternalOutput").ap()

    with tile.TileContext(nc) as tc, ExitStack() as stk:
        def pool(**kw):
            return stk.enter_context(tc.tile_pool(**kw))

        P = {
            "pW": pool(name="pW", bufs=1),
            "pP": pool(name="pP", bufs=1),
            "pA": pool(name="pA", bufs=3),
            "pB": pool(name="pB", bufs=2),
            "pC": pool(name="pC", bufs=1),
            "psS": pool(name="psS", bufs=1, space="PSUM"),
            "psT": pool(name="psT", bufs=2, space="PSUM"),
            "psC": pool(name="psC", bufs=1, space="PSUM"),
        }

        # --- weights / constants: loaded once, read by every rep ----------
        pW = P["pW"]
        wqt_sb = pW.tile([DQ, H, DQ], F32, tag="wqt_sb", name="wqt_sb")
        nc.sync.dma_start(wqt_sb[:], wqt[:])
        wkt_sb = pW.tile([KHW, KCH, DK], F32, tag="wkt_sb", name="wkt_sb")
        wvb_sb = pW.tile([KHW, KCH, DK], BF16, tag="wvb_sb", name="wvb_sb")
        for j in range(KCH):
            nc.sync.dma_start(wkt_sb[:, j, :], wkt[j * KHW:(j + 1) * KHW, :])
            nc.sync.dma_start(wvb_sb[:, j, :], wvb[j * KHW:(j + 1) * KHW, :])
        wotb_sb = pW.tile([DQ, H, C], BF16, tag="wotb_sb", name="wotb_sb")
        nc.sync.dma_start(wotb_sb[:], wotb[:])
        idb_sb = pW.tile([PT, PT], BF16, tag="idb_sb", name="idb_sb")
        nc.sync.dma_start(idb_sb[:], idb[:])
        ones_sb = pW.tile([PT, PT], F32, tag="ones_sb", name="ones_sb")
        nc.vector.memset(ones_sb[:], 1.0)
        magic_sb = pW.tile([DQ, H], I32, tag="magic_sb", name="magic_sb")
        nc.vector._memset_packed(magic_sb[:], RSQRT_MAGIC)

        W = (wqt_sb, wkt_sb, wvb_sb, wotb_sb, idb_sb, ones_sb, magic_sb)
        for _ in range(reps):
            build_body(tc, P, W, e1, ea, o1)
    # Legalize: walrus encodes at most one sync-wait command per instruction;
    # bacc's compile() splits excess waits into event-semaphore instructions.
    nc.compile()
    return nc


def build_body(tc, P, W, e1, ea, o1):
    nc = tc.nc
    wqt_sb, wkt_sb, wvb_sb, wotb_sb, idb_sb, ones_sb, magic_sb = W
    pP, pA, pB, pC = P["pP"], P["pA"], P["pB"], P["pC"]
    psS, psT, psC = P["psS"], P["psT"], P["psC"]

    # Phase B's small PSUM scratch reuses the S-accumulator banks (tags
    # s01/s23): S is dead from its phase-A evacuation until the next rep's
    # phase A, so the WARs this creates are all already-satisfied.
    psb_ctr = [0]

    def psb_tile(shape, dtype, name):
        tag = ("s01", "s23")[psb_ctr[0] % 2]
        psb_ctr[0] += 1
        return psS.tile(shape, dtype, tag=tag, name=name)

    # --- per-rep tiles (tag rotation = cross-rep ping/pong) ---------------
    # bufs=2: still read during rep i's phase C while rep i+1 writes.
    eaT = pP.tile([CHW, DCH, N], BF16, tag="eaT", name="eaT", bufs=2)
    p2t_sb = pP.tile([KHW, H, 2, DQ], BF16, tag="p2t", name="p2t_sb", bufs=2)
    rec_all = pP.tile([DQ, H], F32, tag="rec", name="rec_all", bufs=2)
    # bufs=1: last read early enough in rep i that rep i+1's write can't stall.
    s_sb = pP.tile([DQ, H, DK], F32, tag="s_sb", name="s_sb")
    sc_all = pP.tile([DQ, H, DK], F32, tag="sc_all", name="sc_all")
    e_all = pP.tile([DQ, H, DK], BF16, tag="e_all", name="e_all")
    stats = pP.tile([DQ, H, 2], F32, tag="stats", name="stats")
    den_all = pP.tile([DQ, H], F32, tag="den", name="den_all")

    def small(tag, dtype=F32):
        return pP.tile([DQ, H], dtype, tag=tag, name=tag)

    # --- phase A: bf16 cast loads; S accumulation; eaT transposes ---------
    # S accumulators live in PSUM for the whole phase: two banks, two heads
    # per bank (free-dim padded to 256 so each head slice stays in-bank).
    s01 = psS.tile([DQ, 2, DK], F32, tag="s01", name="s01",
                   padded_shape=[DQ, 2, 256])
    s23 = psS.tile([DQ, 2, DK], F32, tag="s23", name="s23",
                   padded_shape=[DQ, 2, 256])

    def s_ps(h):
        return (s01 if h < 2 else s23)[:, h % 2, :]

    for ii in range(NG):
        # four 128-row tiles per DMA: row a*128+p of the group lands on
        # partition p, free-slot a (fewer, larger DMA transfers)
        prows = slice(ii * GRP * PT, (ii + 1) * GRP * PT)
        e1b = pA.tile([PT, GRP, C], BF16, tag="e1b", name="e1b")
        nc.gpsimd.dma_start(
            e1b[:], e1[prows, :].rearrange("(a p) k -> p a k", p=PT))
        eab = pA.tile([PT, GRP, KV], BF16, tag="eab", name="eab")
        nc.gpsimd.dma_start(
            eab[:], ea[prows, :].rearrange("(a p) k -> p a k", p=PT))
        for a in range(GRP):
            i = GRP * ii + a
            for h in range(H):
                # Each bank holds TWO heads, and a PSUM bank admits only one
                # accumulation group at a time: open the group on the bank's
                # first matmul (even head, i=0) -- start zeroes the whole 2KB
                # zero region, covering the odd head too -- and close it on
                # the bank's last matmul (odd head, i=NT-1).
                nc.tensor.matmul(
                    s_ps(h),
                    e1b[:, a, h * DQ:(h + 1) * DQ],
                    eab[:, a, h * DK:(h + 1) * DK],
                    start=(i == 0 and h % 2 == 0),
                    stop=(i == NT - 1 and h % 2 == 1),
                )
        # transpose chunk PAIRS into one PSUM bank (2 x [120, 512] halves),
        # then ONE [120, 2, 512] evacuation per pair -- 32 wide evacs per
        # rep instead of 256 narrow ones, alternating DVE/ACT.  psT bufs=2
        # keeps two pairs in flight so transposes never wait on evacs.
        for jp in range(DCH // 2):
            tpt = psT.tile([CHW, 2, GRP, PT], BF16, tag="tp", name="tpt")
            for jj in range(2):
                j = 2 * jp + jj
                for a in range(GRP):
                    nc.tensor.transpose(
                        tpt[:, jj, a, :], eab[:, a, j * CHW:(j + 1) * CHW],
                        idb_sb[:])
            dst = eaT[:, 2 * jp:2 * jp + 2, ii * GRP * PT:(ii + 1) * GRP * PT]
            if (ii * (DCH // 2) + jp) % 2 == 0:
                nc.vector.tensor_copy(dst, tpt[:])
            else:
                nc.scalar.copy(dst, tpt[:])
    for h in range(H):
        nc.vector.tensor_copy(s_sb[:, h, :], s_ps(h))

    # --- phase B ----------------------------------------------------------
    # Staged across heads (all U matmuls, then all scores, ...) so the
    # per-head math chains pipeline 2-deep through the alternating psb banks.
    ut_sbs = []
    for h in range(H):
        # U.T = (S.T-chunks) @ (Wq_h.T/sqrt(KV))  [240k, 128e], fp32
        ut_sb = pB.tile([KHW, KCH, DQ], F32, tag="ut", name="ut_sb", bufs=4)
        for j in range(KCH):
            ut_ps = psb_tile([KHW, DQ], F32, "ut_ps")
            nc.tensor.matmul(ut_ps[:], s_sb[:, h, j * KHW:(j + 1) * KHW],
                             wqt_sb[:, h, :], start=True, stop=True)
            nc.vector.tensor_copy(ut_sb[:, j, :], ut_ps[:])
        ut_sbs.append(ut_sb)
    for h in range(H):
        # scores = U @ Wk.T  [128e, 240ek], fp32
        sc_ps = psb_tile([DQ, DK], F32, "sc_ps")
        for j in range(KCH):
            nc.tensor.matmul(sc_ps[:], ut_sbs[h][:, j, :], wkt_sb[:, j, :],
                             start=(j == 0), stop=(j == KCH - 1))
        # evacuate + per-row sums of x and x^2 for instance-norm
        nc.scalar.activation(sc_all[:, h, :], sc_ps[:], AF.Copy,
                             accum_out=stats[:, h, 0:1])
        junk = pB.tile([DQ, DK], F32, tag="junk", name="junk", bufs=1)
        nc.scalar.activation(junk[:], sc_ps[:], AF.Square,
                             accum_out=stats[:, h, 1:2])

    # cross-partition reduce of stats; every partition gets the totals
    tot_ps = psb_tile([DQ, H, 2], F32, "tot_ps")
    nc.tensor.matmul(tot_ps[:], ones_sb[:], stats[:], start=True, stop=True)
    # var + eps on DVE, then rstd via the bit-trick rsqrt (2 Newton steps,
    # rel err ~1e-6).  Keeps Sqrt off ACT so its LUT stays on
    # exp_and_others (Copy/Square/Exp) -- no per-rep table reloads.
    mu = small("mu")
    nc.vector.tensor_scalar_mul(mu[:], tot_ps[:, :, 0], 1.0 / NORM_CNT)
    mu2 = small("mu2")
    nc.vector.tensor_mul(mu2[:], mu[:], mu[:])
    m2e = small("m2e")
    nc.vector.tensor_scalar(m2e[:], tot_ps[:, :, 1], 1.0 / NORM_CNT, EPS,
                            op0=ALU.mult, op1=ALU.add)
    vare = small("vare")
    nc.vector.tensor_sub(vare[:], m2e[:], mu2[:])
    ihalf = small("ihalf", I32)
    nc.vector.tensor_scalar(ihalf[:], vare[:].bitcast(I32), 1, None,
                            op0=ALU.logical_shift_right)
    y0 = small("y0")
    nc.vector.tensor_sub(y0[:].bitcast(I32), magic_sb[:], ihalf[:])
    y = y0
    for it in range(2):
        t1 = small(f"t1_{it}")
        nc.vector.tensor_mul(t1[:], y[:], y[:])
        t2 = small(f"t2_{it}")
        nc.vector.tensor_mul(t2[:], vare[:], t1[:])
        t3 = small(f"t3_{it}")
        nc.vector.tensor_scalar(t3[:], t2[:], -0.5, 1.5,
                                op0=ALU.mult, op1=ALU.add)
        yn = small(f"y_{it + 1}")
        nc.vector.tensor_mul(yn[:], y[:], t3[:])
        y = yn
    rstd_all = y

    # softmax over ek of rstd*scores: the mean shift cancels in softmax,
    # and no max-shift is needed -- scores are z-scored by rstd so
    # |exponent| stays ~<=8, far from fp32 overflow.  The 1/denominator
    # is applied later as the ctx-evac scale.
    for h in range(H):
        nc.scalar.activation(e_all[:, h, :], sc_all[:, h, :],
                             AF.Exp, scale=rstd_all[:, h:h + 1],
                             accum_out=den_all[:, h:h + 1])
    nc.vector.reciprocal(rec_all[:], den_all[:])
    # P2.T = (exp @ Wv).T in chunks: [120d, 128c]  (unnormalized)
    pt_sbs = []
    for h in range(H):
        pt_sb = pB.tile([KHW, KCH, DQ], BF16, tag="pt", name="pt_sb", bufs=4)
        for j in range(KCH):
            pt_ps = psb_tile([KHW, DQ], BF16, "pt_ps")
            nc.tensor.transpose(
                pt_ps[:], e_all[:, h, j * KHW:(j + 1) * KHW], idb_sb[:])
            nc.vector.tensor_copy(pt_sb[:, j, :], pt_ps[:])
        pt_sbs.append(pt_sb)
    for h in range(H):
        for jd in range(2):
            p2t_ps = psb_tile([CHW, DQ], F32, "p2t_ps")
            for jk in range(KCH):
                nc.tensor.matmul(
                    p2t_ps[:],
                    wvb_sb[:, jk, jd * CHW:(jd + 1) * CHW],
                    pt_sbs[h][:, jk, :],
                    start=(jk == 0), stop=(jk == KCH - 1))
            nc.scalar.copy(p2t_sb[:, h, jd, :], p2t_ps[:])

    # --- phase C: ctx_h = P2_h @ Kc_h.T, then O1 = sum_h ctx_h.T @ WoT_h --
    # Software-pipelined: the context matmuls for chunk n+1 are emitted
    # before chunk n's output projection, so the PE never stalls on the
    # context evacuations.
    # cx_ps and o_ps share one 4-deep PSUM rotation (their lifetimes
    # interleave 4+4 per chunk), so each tile's WAR reaches 4 allocations
    # back instead of 2 -- enough slack to hide the evac+sem latencies.
    def emit_ctx(nch):
        ncols = slice(nch * 512, (nch + 1) * 512)
        ctx = pC.tile([DQ, H, 512], BF16, tag="ctx", name="ctx", bufs=2)
        for h in range(H):
            cx_ps = psC.tile([DQ, 512], F32, tag="big", name="cx_ps", bufs=4)
            for jd in range(2):
                nc.tensor.matmul(cx_ps[:], p2t_sb[:, h, jd, :],
                                 eaT[:, 2 * h + jd, ncols],
                                 start=(jd == 0), stop=(jd == 1))
            # normalize: rows of ctx are q-channels, so the softmax
            # denominator applies as a per-partition scale here.
            nc.vector.tensor_scalar_mul(ctx[:, h, :], cx_ps[:],
                                        rec_all[:, h:h + 1])
        return ctx

    def emit_oproj(nch, ctx):
        for t in range(4):
            i = nch * 4 + t
            rows = slice(i * PT, (i + 1) * PT)
            o_ps = psC.tile([PT, C], F32, tag="big", name="o_ps", bufs=4)
            for h in range(H):
                nc.tensor.matmul(o_ps[:],
                                 ctx[:, h, t * PT:(t + 1) * PT],
                                 wotb_sb[:, h, :],
                                 start=(h == 0), stop=(h == H - 1))
            o_sb = pC.tile([PT, C], F32, tag="o_sb", name="o_sb", bufs=3)
            nc.scalar.copy(o_sb[:], o_ps[:])
            nc.sync.dma_start(o1[rows, :], o_sb[:])

    prev = emit_ctx(0)
    for nch in range(1, NCH):
        cur = emit_ctx(nch)
        emit_oproj(nch - 1, prev)
        prev = cur
    emit_oproj(NCH - 1, prev)


_NC_CACHE = None


def get_nc():
    global _NC_CACHE
    if _NC_CACHE is None:
        _NC_CACHE = build_nc()
    return _NC_CACHE


def make_in_maps(emb1, emb_all, Wq, Wk, Wv, Wo):
    emb1 = np.ascontiguousarray(np.asarray(emb1, dtype=np.float32))
    emb_all = np.ascontiguousarray(np.asarray(emb_all, dtype=np.float32))
    Wq = np.asarray(Wq, dtype=np.float32)
    Wk = np.asarray(Wk, dtype=np.float32)
    Wv = np.asarray(Wv, dtype=np.float32)
    Wo = np.asarray(Wo, dtype=np.float32)

    scale = 1.0 / np.sqrt(np.float32(KV))
    wqt_np = np.ascontiguousarray(np.transpose(Wq, (2, 0, 1)) * scale)  # [c,h,e]
    wkt_np = np.ascontiguousarray(Wk.T)                                 # [k,ek]
    wvb_np = np.ascontiguousarray(Wv).astype(ml_dtypes.bfloat16)        # [k,d]
    wotb_np = np.ascontiguousarray(
        Wo.reshape(C, DQ, H).transpose(1, 2, 0)).astype(ml_dtypes.bfloat16)
    idb_np = np.eye(PT, dtype=ml_dtypes.bfloat16)

    shared = {"wqt": wqt_np, "wkt": wkt_np, "wvb": wvb_np, "wotb": wotb_np,
              "idb": idb_np}
    return [
        {"e1": emb1[b], "ea": emb_all[b], **shared}
        for b in range(B)
    ]


def run(inputs, trace=False, **spmd_kwargs):
    nc = get_nc()
    in_maps = make_in_maps(**inputs)
    res = run_bass_kernel_spmd(nc, in_maps, list(range(B)), trace=trace,
                               **spmd_kwargs)
    out = np.stack([np.asarray(res.results[b]["o1"]) for b in range(B)], axis=0)
    return out.astype(np.float32, copy=False), res


def kernel(**inputs) -> np.ndarray:
    out, _ = run(inputs, trace=False)
    return out


# revision 16
# speedup vs baseline: 2.0075x; 1.4857x over previous
"""Trainium2 Bass kernel for nn_Attention_org_single_85074712199391.

Channel-attention module. Reference math (per batch b, head h):
    Qc = emb1[b].reshape(N, 4, dq)[:, h]          # [N, 128]
    Kc = emb_all[b].reshape(N, 4, dk)[:, h]       # [N, 240]
    Q = Qc @ Wq[h].T ; K = Kc @ Wk.T ; V = Kc @ Wv.T
    scores = Q.T @ K / sqrt(KV)                   # [128, 240]
    probs = softmax(instnorm(scores), axis=-1)
    context = probs @ V.T                         # [128, N]
    O1 = permute/concat(context) @ Wo.T           # [N, 512]

Algebraic rewrite used here (exact):
    S_h      = Qc.T @ Kc                          # big contraction over N
    scores_h = (Wq[h]/sqrt(KV)) @ S_h @ Wk.T
    probs_h  = softmax over dk of rstd*scores_h   # mean cancels in softmax
    P2_h     = probs_h @ Wv                       # [128, 240]
    ctx_h    = P2_h @ Kc.T                        # [128, N]
    O1       = sum_h ctx_h.T @ Wo[:, h::4].T      # accumulate over heads

Sharding: core b owns batch b; weights replicated; no collectives.

Pipelined-reps structure: ALL tile pools are created once at TileContext
scope and shared by every rep body.  Per-rep tiles are requested with a
fixed tag, so the pool's buf rotation gives automatic ping/pong: tensors
that are still being read at the end of rep i (eaT, p2t, rec) use bufs=2,
which makes rep i+1's writes land in the other buffer and lets rep i+1's
load phase stream underneath rep i's compute tail.  Weights are loaded
once, outside the rep loop.

Engine budget per rep (steady state):
    DMA : e1+ea fp32 reads (23.7 MB) + o1 writes (8 MB)  -- the roofline
    PE  : S matmuls, ea transposes, tiny phase B, ctx + output projection
    DVE : half the eaT evacuations, ctx evacuations, phase-B smalls,
          bit-trick rsqrt (keeps ACT pinned to the exp_and_others table)
    ACT : other half of eaT evacuations, o evacuations, stats + exp
    POOL: SWDGE descriptor generation for the cast loads (f32->bf16)
"""

import sys

import numpy as np

try:
    import concourse.bass as bass
except ImportError:  # harness environments without the repo on sys.path
    sys.path.insert(0, "/opt/trn_rl_repo")
    import concourse.bass as bass

import concourse.bacc as bacc

import ml_dtypes
import concourse.mybir as mybir
import concourse.tile as tile
from concourse.bass_utils import run_bass_kernel_spmd

F32 = mybir.dt.float32
BF16 = mybir.dt.bfloat16
I32 = mybir.dt.int32
AF = mybir.ActivationFunctionType
ALU = mybir.AluOpType

B, N, C, KV, H = 8, 4096, 512, 960, 4
DQ, DK = C // 4, KV // 4          # 128, 240
PT = 128                          # partition tile
NT = N // PT                      # 32 row tiles
GRP = 4                           # row tiles per DMA group
NG = NT // GRP                    # 8 groups
DCH = 8                           # KV split into 8 chunks of 120 partitions
CHW = KV // DCH                   # 120
KCH = 2                           # dk split for 240-deep contractions
KHW = DK // KCH                   # 120
NCH = N // 512                    # 8 column chunks for phase C
EPS = 1e-5
NORM_CNT = float(DQ * DK)         # instance-norm element count
RSQRT_MAGIC = 0x5F3759DF


def build_nc(reps=1):
    from contextlib import ExitStack

    nc = bacc.Bacc("TRN2", target_bir_lowering=False, debug=False)

    e1 = nc.dram_tensor("e1", [N, C], F32, kind="ExternalInput").ap()
    ea = nc.dram_tensor("ea", [N, KV], F32, kind="ExternalInput").ap()
    wqt = nc.dram_tensor("wqt", [DQ, H, DQ], BF16, kind="ExternalInput").ap()
    wkt = nc.dram_tensor("wkt", [DK, DK], BF16, kind="ExternalInput").ap()
    wvb = nc.dram_tensor("wvb", [DK, DK], BF16, kind="ExternalInput").ap()
    wotb = nc.dram_tensor("wotb", [DQ, H, C], BF16, kind="ExternalInput").ap()
    idb = nc.dram_tensor("idb", [PT, PT], BF16, kind="ExternalInput").ap()
    o1 = nc.dram_tensor("o1", [N, C], F32, kind="ExternalOutput").ap()

    with tile.TileContext(nc) as tc, ExitStack() as stk:
        def pool(**kw):
            return stk.enter_context(tc.tile_pool(**kw))

        P = {
            "pW": pool(name="pW", bufs=1),
            "pP": pool(name="pP", bufs=1),
            "pA": pool(name="pA", bufs=3),
            "pB": pool(name="pB", bufs=2),
            "pC": pool(name="pC", bufs=1),
            "psS": pool(name="psS", bufs=1, space="PSUM"),
            "psT": pool(name="psT", bufs=2, space="PSUM"),
            "psC": pool(name="psC", bufs=1, space="PSUM"),
        }

        # --- weights / constants: loaded once, read by every rep ----------
        pW = P["pW"]
        wqt_sb = pW.tile([DQ, H, DQ], BF16, tag="wqt_sb", name="wqt_sb")
        nc.sync.dma_start(wqt_sb[:], wqt[:])
        wkt_sb = pW.tile([KHW, KCH, DK], BF16, tag="wkt_sb", name="wkt_sb")
        wvb_sb = pW.tile([KHW, KCH, DK], BF16, tag="wvb_sb", name="wvb_sb")
        for j in range(KCH):
            nc.sync.dma_start(wkt_sb[:, j, :], wkt[j * KHW:(j + 1) * KHW, :])
            nc.sync.dma_start(wvb_sb[:, j, :], wvb[j * KHW:(j + 1) * KHW, :])
        wotb_sb = pW.tile([DQ, H, C], BF16, tag="wotb_sb", name="wotb_sb")
        nc.sync.dma_start(wotb_sb[:], wotb[:])
        idb_sb = pW.tile([PT, PT], BF16, tag="idb_sb", name="idb_sb")
        nc.sync.dma_start(idb_sb[:], idb[:])
        ones_sb = pW.tile([PT, PT], F32, tag="ones_sb", name="ones_sb")
        nc.vector.memset(ones_sb[:], 1.0)
        magic_sb = pW.tile([DQ, H], I32, tag="magic_sb", name="magic_sb")
        nc.vector._memset_packed(magic_sb[:], RSQRT_MAGIC)

        W = (wqt_sb, wkt_sb, wvb_sb, wotb_sb, idb_sb, ones_sb, magic_sb)
        for _ in range(reps):
            build_body(tc, P, W, e1, ea, o1)
    # Legalize: walrus encodes at most one sync-wait command per instruction;
    # bacc's compile() splits excess waits into event-semaphore instructions.
    nc.compile()
    return nc


def build_body(tc, P, W, e1, ea, o1):
    nc = tc.nc
    wqt_sb, wkt_sb, wvb_sb, wotb_sb, idb_sb, ones_sb, magic_sb = W
    pP, pA, pB, pC = P["pP"], P["pA"], P["pB"], P["pC"]
    psS, psT, psC = P["psS"], P["psT"], P["psC"]

    # Phase B's small PSUM scratch reuses the S-accumulator banks (tags
    # s01/s23): S is dead from its phase-A evacuation until the next rep's
    # phase A, so the WARs this creates are all already-satisfied.
    psb_ctr = [0]

    def psb_tile(shape, dtype, name):
        tag = ("s01", "s23")[psb_ctr[0] % 2]
        psb_ctr[0] += 1
        return psS.tile(shape, dtype, tag=tag, name=name)

    # --- per-rep tiles (tag rotation = cross-rep ping/pong) ---------------
    # bufs=2: still read during rep i's phase C while rep i+1 writes.
    eaT = pP.tile([CHW, DCH, N], BF16, tag="eaT", name="eaT", bufs=2)
    p2t_sb = pP.tile([KHW, H, 2, DQ], BF16, tag="p2t", name="p2t_sb", bufs=2)
    rec_all = pP.tile([DQ, H], F32, tag="rec", name="rec_all", bufs=2)
    # bufs=1: last read early enough in rep i that rep i+1's write can't stall.
    s_sb = pP.tile([DQ, H, DK], BF16, tag="s_sb", name="s_sb")
    sc_all = pP.tile([DQ, H, DK], F32, tag="sc_all", name="sc_all")
    e_all = pP.tile([DQ, H, DK], BF16, tag="e_all", name="e_all")
    stats = pP.tile([DQ, H, 2], F32, tag="stats", name="stats")
    den_all = pP.tile([DQ, H], F32, tag="den", name="den_all")

    def small(tag, dtype=F32):
        return pP.tile([DQ, H], dtype, tag=tag, name=tag)

    # --- phase A: bf16 cast loads; S accumulation; eaT transposes ---------
    # S accumulators live in PSUM for the whole phase: two banks, two heads
    # per bank (free-dim padded to 256 so each head slice stays in-bank).
    s01 = psS.tile([DQ, 2, DK], F32, tag="s01", name="s01",
                   padded_shape=[DQ, 2, 256])
    s23 = psS.tile([DQ, 2, DK], F32, tag="s23", name="s23",
                   padded_shape=[DQ, 2, 256])

    def s_ps(h):
        return (s01 if h < 2 else s23)[:, h % 2, :]

    for ii in range(NG):
        # four 128-row tiles per DMA: row a*128+p of the group lands on
        # partition p, free-slot a (fewer, larger DMA transfers)
        prows = slice(ii * GRP * PT, (ii + 1) * GRP * PT)
        e1b = pA.tile([PT, GRP, C], BF16, tag="e1b", name="e1b")
        nc.gpsimd.dma_start(
            e1b[:], e1[prows, :].rearrange("(a p) k -> p a k", p=PT))
        eab = pA.tile([PT, GRP, KV], BF16, tag="eab", name="eab")
        nc.gpsimd.dma_start(
            eab[:], ea[prows, :].rearrange("(a p) k -> p a k", p=PT))
        for a in range(GRP):
            i = GRP * ii + a
            for h in range(H):
                # Each bank holds TWO heads, and a PSUM bank admits only one
                # accumulation group at a time: open the group on the bank's
                # first matmul (even head, i=0) -- start zeroes the whole 2KB
                # zero region, covering the odd head too -- and close it on
                # the bank's last matmul (odd head, i=NT-1).
                nc.tensor.matmul(
                    s_ps(h),
                    e1b[:, a, h * DQ:(h + 1) * DQ],
                    eab[:, a, h * DK:(h + 1) * DK],
                    start=(i == 0 and h % 2 == 0),
                    stop=(i == NT - 1 and h % 2 == 1),
                )
        # transpose chunk PAIRS into one PSUM bank (2 x [120, 512] halves),
        # then ONE [120, 2, 512] evacuation per pair -- 32 wide evacs per
        # rep instead of 256 narrow ones, alternating DVE/ACT.  psT bufs=2
        # keeps two pairs in flight so transposes never wait on evacs.
        for jp in range(DCH // 2):
            tpt = psT.tile([CHW, 2, GRP, PT], BF16, tag="tp", name="tpt")
            for jj in range(2):
                j = 2 * jp + jj
                for a in range(GRP):
                    nc.tensor.transpose(
                        tpt[:, jj, a, :], eab[:, a, j * CHW:(j + 1) * CHW],
                        idb_sb[:])
            dst = eaT[:, 2 * jp:2 * jp + 2, ii * GRP * PT:(ii + 1) * GRP * PT]
            # 2:1 DVE:ACT split -- ACT carries the o-evacuations and stats
            if (ii * (DCH // 2) + jp) % 3 != 2:
                nc.vector.tensor_copy(dst, tpt[:])
            else:
                nc.scalar.copy(dst, tpt[:])
    for h in range(H):
        nc.vector.tensor_copy(s_sb[:, h, :], s_ps(h))

    # --- phase B ----------------------------------------------------------
    # Staged across heads (all U matmuls, then all scores, ...) so the
    # per-head math chains pipeline 2-deep through the alternating psb banks.
    ut_sbs = []
    for h in range(H):
        # U.T = (S.T-chunks) @ (Wq_h.T/sqrt(KV))  [240k, 128e], fp32
        ut_sb = pB.tile([KHW, KCH, DQ], BF16, tag="ut", name="ut_sb", bufs=4)
        for j in range(KCH):
            ut_ps = psb_tile([KHW, DQ], F32, "ut_ps")
            nc.tensor.matmul(ut_ps[:], s_sb[:, h, j * KHW:(j + 1) * KHW],
                             wqt_sb[:, h, :], start=True, stop=True)
            nc.vector.tensor_copy(ut_sb[:, j, :], ut_ps[:])
        ut_sbs.append(ut_sb)
    for h in range(H):
        # scores = U @ Wk.T  [128e, 240ek], fp32
        sc_ps = psb_tile([DQ, DK], F32, "sc_ps")
        for j in range(KCH):
            nc.tensor.matmul(sc_ps[:], ut_sbs[h][:, j, :], wkt_sb[:, j, :],
                             start=(j == 0), stop=(j == KCH - 1))
        # evacuate + per-row sums of x and x^2 for instance-norm
        nc.scalar.activation(sc_all[:, h, :], sc_ps[:], AF.Copy,
                             accum_out=stats[:, h, 0:1])
        junk = pB.tile([DQ, DK], F32, tag="junk", name="junk", bufs=1)
        nc.scalar.activation(junk[:], sc_ps[:], AF.Square,
                             accum_out=stats[:, h, 1:2])

    # cross-partition reduce of stats; every partition gets the totals
    tot_ps = psb_tile([DQ, H, 2], F32, "tot_ps")
    nc.tensor.matmul(tot_ps[:], ones_sb[:], stats[:], start=True, stop=True)
    # var + eps on DVE, then rstd via the bit-trick rsqrt (2 Newton steps,
    # rel err ~1e-6).  Keeps Sqrt off ACT so its LUT stays on
    # exp_and_others (Copy/Square/Exp) -- no per-rep table reloads.
    mu = small("mu")
    nc.vector.tensor_scalar_mul(mu[:], tot_ps[:, :, 0], 1.0 / NORM_CNT)
    mu2 = small("mu2")
    nc.vector.tensor_mul(mu2[:], mu[:], mu[:])
    m2e = small("m2e")
    nc.vector.tensor_scalar(m2e[:], tot_ps[:, :, 1], 1.0 / NORM_CNT, EPS,
                            op0=ALU.mult, op1=ALU.add)
    vare = small("vare")
    nc.vector.tensor_sub(vare[:], m2e[:], mu2[:])
    ihalf = small("ihalf", I32)
    nc.vector.tensor_scalar(ihalf[:], vare[:].bitcast(I32), 1, None,
                            op0=ALU.logical_shift_right)
    y0 = small("y0")
    nc.vector.tensor_sub(y0[:].bitcast(I32), magic_sb[:], ihalf[:])
    y = y0
    for it in range(2):
        t1 = small(f"t1_{it}")
        nc.vector.tensor_mul(t1[:], y[:], y[:])
        t2 = small(f"t2_{it}")
        nc.vector.tensor_mul(t2[:], vare[:], t1[:])
        t3 = small(f"t3_{it}")
        nc.vector.tensor_scalar(t3[:], t2[:], -0.5, 1.5,
                                op0=ALU.mult, op1=ALU.add)
        yn = small(f"y_{it + 1}")
        nc.vector.tensor_mul(yn[:], y[:], t3[:])
        y = yn
    rstd_all = y

    # softmax over ek of rstd*scores: the mean shift cancels in softmax,
    # and no max-shift is needed -- scores are z-scored by rstd so
    # |exponent| stays ~<=8, far from fp32 overflow.  The 1/denominator
    # is applied later as the ctx-evac scale.
    for h in range(H):
        nc.scalar.activation(e_all[:, h, :], sc_all[:, h, :],
                             AF.Exp, scale=rstd_all[:, h:h + 1],
                             accum_out=den_all[:, h:h + 1])
    nc.vector.reciprocal(rec_all[:], den_all[:])
    # P2.T = (exp @ Wv).T in chunks: [120d, 128c]  (unnormalized)
    pt_sbs = []
    for h in range(H):
        pt_sb = pB.tile([KHW, KCH, DQ], BF16, tag="pt", name="pt_sb", bufs=4)
        for j in range(KCH):
            pt_ps = psb_tile([KHW, DQ], BF16, "pt_ps")
            nc.tensor.transpose(
                pt_ps[:], e_all[:, h, j * KHW:(j + 1) * KHW], idb_sb[:])
            nc.vector.tensor_copy(pt_sb[:, j, :], pt_ps[:])
        pt_sbs.append(pt_sb)
    for h in range(H):
        for jd in range(2):
            p2t_ps = psb_tile([CHW, DQ], F32, "p2t_ps")
            for jk in range(KCH):
                nc.tensor.matmul(
                    p2t_ps[:],
                    wvb_sb[:, jk, jd * CHW:(jd + 1) * CHW],
                    pt_sbs[h][:, jk, :],
                    start=(jk == 0), stop=(jk == KCH - 1))
            nc.scalar.copy(p2t_sb[:, h, jd, :], p2t_ps[:])

    # --- phase C: ctx_h = P2_h @ Kc_h.T, then O1 = sum_h ctx_h.T @ WoT_h --
    # Software-pipelined: the context matmuls for chunk n+1 are emitted
    # before chunk n's output projection, so the PE never stalls on the
    # context evacuations.
    # cx_ps and o_ps share one 4-deep PSUM rotation (their lifetimes
    # interleave 4+4 per chunk), so each tile's WAR reaches 4 allocations
    # back instead of 2 -- enough slack to hide the evac+sem latencies.
    def emit_ctx(nch):
        ncols = slice(nch * 512, (nch + 1) * 512)
        ctx = pC.tile([DQ, H, 512], BF16, tag="ctx", name="ctx", bufs=2)
        for h in range(H):
            cx_ps = psC.tile([DQ, 512], F32, tag="big", name="cx_ps", bufs=4)
            for jd in range(2):
                nc.tensor.matmul(cx_ps[:], p2t_sb[:, h, jd, :],
                                 eaT[:, 2 * h + jd, ncols],
                                 start=(jd == 0), stop=(jd == 1))
            # normalize: rows of ctx are q-channels, so the softmax
            # denominator applies as a per-partition scale here.
            nc.vector.tensor_scalar_mul(ctx[:, h, :], cx_ps[:],
                                        rec_all[:, h:h + 1])
        return ctx

    def emit_oproj(nch, ctx):
        # two n-tiles share one o_sb staging tile -> 16 stores of 512 KB
        # instead of 32 of 256 KB (better sustained store bandwidth)
        for t in range(4):
            i = nch * 4 + t
            if t % 2 == 0:
                o_sb = pC.tile([PT, 2, C], F32, tag="o_sb", name="o_sb",
                               bufs=2)
            o_ps = psC.tile([PT, C], F32, tag="big", name="o_ps", bufs=4)
            for h in range(H):
                nc.tensor.matmul(o_ps[:],
                                 ctx[:, h, t * PT:(t + 1) * PT],
                                 wotb_sb[:, h, :],
                                 start=(h == 0), stop=(h == H - 1))
            nc.scalar.copy(o_sb[:, t % 2, :], o_ps[:])
            if t % 2 == 1:
                rows2 = slice((i - 1) * PT, (i + 1) * PT)
                nc.sync.dma_start(
                    o1[rows2, :].rearrange("(b p) c -> p b c", p=PT), o_sb[:])

    prev = emit_ctx(0)
    for nch in range(1, NCH):
        cur = emit_ctx(nch)
        emit_oproj(nch - 1, prev)
        prev = cur
    emit_oproj(NCH - 1, prev)


_NC_CACHE = None


def get_nc():
    global _NC_CACHE
    if _NC_CACHE is None:
        _NC_CACHE = build_nc()
    return _NC_CACHE


def make_in_maps(emb1, emb_all, Wq, Wk, Wv, Wo):
    emb1 = np.ascontiguousarray(np.asarray(emb1, dtype=np.float32))
    emb_all = np.ascontiguousarray(np.asarray(emb_all, dtype=np.float32))
    Wq = np.asarray(Wq, dtype=np.float32)
    Wk = np.asarray(Wk, dtype=np.float32)
    Wv = np.asarray(Wv, dtype=np.float32)
    Wo = np.asarray(Wo, dtype=np.float32)

    scale = 1.0 / np.sqrt(np.float32(KV))
    wqt_np = np.ascontiguousarray(
        np.transpose(Wq, (2, 0, 1)) * scale).astype(ml_dtypes.bfloat16)
    wkt_np = np.ascontiguousarray(Wk.T).astype(ml_dtypes.bfloat16)      # [k,ek]
    wvb_np = np.ascontiguousarray(Wv).astype(ml_dtypes.bfloat16)        # [k,d]
    wotb_np = np.ascontiguousarray(
        Wo.reshape(C, DQ, H).transpose(1, 2, 0)).astype(ml_dtypes.bfloat16)
    idb_np = np.eye(PT, dtype=ml_dtypes.bfloat16)

    shared = {"wqt": wqt_np, "wkt": wkt_np, "wvb": wvb_np, "wotb": wotb_np,
              "idb": idb_np}
    return [
        {"e1": emb1[b], "ea": emb_all[b], **shared}
        for b in range(B)
    ]


def run(inputs, trace=False, **spmd_kwargs):
    nc = get_nc()
    in_maps = make_in_maps(**inputs)
    res = run_bass_kernel_spmd(nc, in_maps, list(range(B)), trace=trace,
                               **spmd_kwargs)
    out = np.stack([np.asarray(res.results[b]["o1"]) for b in range(B)], axis=0)
    return out.astype(np.float32, copy=False), res


def kernel(**inputs) -> np.ndarray:
    out, _ = run(inputs, trace=False)
    return out
